# revision 7
# baseline (speedup 1.0000x reference)
"""Trainium2 Bass kernel for nn_Block (BitNet-style quantized transformer block).

Sharding: 8 cores; core c handles batch b=c//2, token half h=c%2 (1024 tokens).
Each core gets the full batch-b tokens (xkv, for K/V) plus its query half (xq),
and replicated host-pre-quantized ternary weights (bf16, exact small ints).
No cross-core communication.

Numerics:
  - weight quant (ternary absmean) is input preprocessing: done host-side,
    exact; per-weight absmean scales are baked into the program as immediates.
  - act_quant produces int8-valued bf16 (exact); activation matmuls are exact
    integer arithmetic accumulated in fp32 PSUM.
  - kT keeps raw int8 K values (token scales folded into the exp scale);
    q is dequantized to real bf16 before scores. Softmax exp runs unshifted
    (scores are O(10), far inside fp32/bf16 range); attn weights and V are
    bf16 (~0.4% rounding, below the model's own int8 quantization noise).
  - round() matches jnp.round exactly (RNE) via the +1.5*2^23 magic trick.

Engine placement (measured: GpSimd tensor_scalar with subtract/min/max ALU ops
hits a ~10-40x slow ucode path and locks the DVE-shared SBUF port, so GpSimd
only runs (mult,add) fast-path quant steps; clamps/casts go to Vector; exp /
gelu / dequant copies to Scalar; LN stats and reduces to Vector).
"""

import sys

sys.path.insert(0, "/opt/trn_rl_repo")

from contextlib import ExitStack

import numpy as np

import concourse.bass as bass
import concourse.bacc as bacc
import concourse.tile as tile
import concourse.mybir as mybir
from concourse.bass_utils import run_bass_kernel_spmd

F32 = mybir.dt.float32
BF16 = mybir.dt.bfloat16
AF = mybir.ActivationFunctionType
ALU = mybir.AluOpType
AX = mybir.AxisListType

DIM = 384
HEADS = 6
HD = 64
HIDDEN = 1536
NKV = 2048
NQ = 1024
CKV = NKV // 128    # 16
CQ = NQ // 128      # 8
IC = DIM // 128     # 3
CH = HIDDEN // 128  # 12
G = 4               # LN/quant group size (chunks)
MAGIC = float(np.float32(1.5 * 2 ** 23))
EPS = 1e-5
ATT_SCALE = HD ** -0.5


def build_program(meta):
    nc = bacc.Bacc("TRN2", target_bir_lowering=False)

    m_qkv = meta["m_qkv"]
    m_proj = meta["m_proj"]
    m_fc1 = meta["m_fc1"]
    m_fc2 = meta["m_fc2"]

    xq_d = nc.dram_tensor("xq", [NQ, DIM], F32, kind="ExternalInput")
    xkv_d = nc.dram_tensor("xkv", [NKV, DIM], F32, kind="ExternalInput")
    wqkvT_d = nc.dram_tensor("wqkvT", [DIM, 3 * DIM], BF16, kind="ExternalInput")
    wprojT_d = nc.dram_tensor("wprojT", [DIM, DIM], BF16, kind="ExternalInput")
    wfc1T_d = nc.dram_tensor("wfc1T", [DIM, HIDDEN], BF16, kind="ExternalInput")
    wfc2T_d = nc.dram_tensor("wfc2T", [HIDDEN, DIM], BF16, kind="ExternalInput")
    eye_d = nc.dram_tensor("eye", [128, 128], F32, kind="ExternalInput")
    out_d = nc.dram_tensor("out", [NQ, DIM], F32, kind="ExternalOutput")

    with tile.TileContext(nc) as tc, ExitStack() as ctx:
        EV, EG, ES = nc.vector, nc.gpsimd, nc.scalar
        pers = ctx.enter_context(tc.tile_pool(name="pers", bufs=1))
        tmp = ctx.enter_context(tc.tile_pool(name="tmp", bufs=3))

        eye = pers.tile([128, 128], F32, tag="eye")
        nc.sync.dma_start(eye[:], eye_d[:])
        eps_t = pers.tile([128, 1], F32, tag="eps")
        EV.memset(eps_t[:], EPS)

        # ---------- pre-quantized ternary weights (bf16, exact ints) -------
        w_qkv = pers.tile([128, IC, 3 * DIM], BF16, tag="w_qkv")
        nc.sync.dma_start(w_qkv[:], wqkvT_d[:].rearrange("(c p) o -> p c o", p=128))
        w_proj = pers.tile([128, IC, DIM], BF16, tag="w_proj")
        nc.sync.dma_start(w_proj[:], wprojT_d[:].rearrange("(c p) o -> p c o", p=128))
        w_fc1 = pers.tile([128, IC, HIDDEN], BF16, tag="w_fc1")
        nc.sync.dma_start(w_fc1[:], wfc1T_d[:].rearrange("(c p) o -> p c o", p=128))
        w_fc2 = pers.tile([128, CH, DIM], BF16, tag="w_fc2")
        nc.sync.dma_start(w_fc2[:], wfc2T_d[:].rearrange("(c p) o -> p c o", p=128))

        # ---------- optional affine/bias tensors (usually compiled out) ----
        def bcast_row(dram_ap, n, name):
            t = pers.tile([128, n], F32, tag=name)
            src = bass.AP(tensor=dram_ap.tensor, offset=dram_ap.offset,
                          ap=[[0, 128]] + list(dram_ap.ap))
            nc.sync.dma_start(t[:], src)
            return t

        ln1_wt = ln1_bt = ln2_wt = ln2_bt = None
        proj_bt = fc1_bt = fc2_bt = None
        if not meta["ln1_trivial"]:
            ln1_wt = bcast_row(nc.dram_tensor("ln1_w", [DIM], F32, kind="ExternalInput")[:], DIM, "ln1w")
            ln1_bt = bcast_row(nc.dram_tensor("ln1_b", [DIM], F32, kind="ExternalInput")[:], DIM, "ln1b")
        if not meta["ln2_trivial"]:
            ln2_wt = bcast_row(nc.dram_tensor("ln2_w", [DIM], F32, kind="ExternalInput")[:], DIM, "ln2w")
            ln2_bt = bcast_row(nc.dram_tensor("ln2_b", [DIM], F32, kind="ExternalInput")[:], DIM, "ln2b")
        assert meta["qkv_b_zero"], "qkv bias not supported in fast path"
        if not meta["proj_b_zero"]:
            proj_bt = bcast_row(nc.dram_tensor("proj_b", [DIM], F32, kind="ExternalInput")[:], DIM, "projb")
        if not meta["fc1_b_zero"]:
            fc1_bt = bcast_row(nc.dram_tensor("fc1_b", [HIDDEN], F32, kind="ExternalInput")[:], HIDDEN, "fc1b")
        if not meta["fc2_b_zero"]:
            fc2_bt = bcast_row(nc.dram_tensor("fc2_b", [DIM], F32, kind="ExternalInput")[:], DIM, "fc2b")

        I32 = mybir.dt.int32
        qmagic = pers.tile([128, 1], I32, tag="qmagic")
        EV.memset(qmagic[:], 0x5F3759DF)

        def rsqrt_dve(out_ap, var_ap, gn):
            # out = 1/sqrt(var + eps) computed entirely on VectorE
            t = tmp.tile([128, G], F32, tag="rs_t")
            EV.tensor_scalar_add(out=t[:, :gn], in0=var_ap, scalar1=EPS)
            y = tmp.tile([128, G], F32, tag="rs_y")
            yi = y[:, :gn].bitcast(I32)
            EV.tensor_scalar(out=yi, in0=t[:, :gn].bitcast(I32),
                             scalar1=1, scalar2=0,
                             op0=ALU.arith_shift_right, op1=ALU.bypass)
            EV.tensor_tensor(yi, qmagic[:, 0:1].to_broadcast((128, gn)), yi,
                             op=ALU.subtract)
            a = tmp.tile([128, G], F32, tag="rs_a")
            for _ in range(3):
                EV.tensor_tensor(a[:, :gn], y[:, :gn], y[:, :gn], op=ALU.mult)
                EV.tensor_tensor(a[:, :gn], a[:, :gn], t[:, :gn], op=ALU.mult)
                EV.tensor_scalar(out=a[:, :gn], in0=a[:, :gn],
                                 scalar1=-0.5, scalar2=1.5,
                                 op0=ALU.mult, op1=ALU.add)
                EV.tensor_tensor(y[:, :gn], y[:, :gn], a[:, :gn], op=ALU.mult)
            EV.tensor_copy(out=out_ap, in_=y[:, :gn])

        # ---------- streaming LN + act_quant (one group of <=G chunks) -----
        def ln_quant_group(src_all, g0, gn, rinv, ln_pool, ln_w, ln_b,
                           use_act=True, post_group=None):
            mv = tmp.tile([128, G, 2], F32, tag="ln_mv")
            for i in range(gn):
                st = tmp.tile([128, 6], F32, tag="ln_bnst")
                EV.bn_stats(out=st[:], in_=src_all[:, g0 + i, :])
                EV.bn_aggr(out=mv[:, i, :], in_=st[:])
            rstd = tmp.tile([128, G], F32, tag="ln_rstd")
            if use_act:
                std = tmp.tile([128, G], F32, tag="ln_std")
                ES.activation(out=std[:, :gn], in_=mv[:, :gn, 1],
                              func=AF.Sqrt, bias=eps_t[:])
                EV.reciprocal(out=rstd[:, :gn], in_=std[:, :gn])
            else:
                rsqrt_dve(rstd[:, :gn], mv[:, :gn, 1], gn)
            nmr = tmp.tile([128, G], F32, tag="ln_nmr")
            EV.tensor_tensor(nmr[:, :gn], mv[:, :gn, 0], rstd[:, :gn],
                             op=ALU.mult)
            EV.tensor_scalar_mul(out=nmr[:, :gn], in0=nmr[:, :gn],
                                 scalar1=-1.0)
            lns = []
            am = tmp.tile([128, G], F32, tag="ln_am")
            for i in range(gn):
                ln = ln_pool.tile([128, DIM], F32, tag=f"ln_{i}")
                if use_act:
                    ES.activation(
                        out=ln[:], in_=src_all[:, g0 + i, :], func=AF.Identity,
                        scale=rstd[:, i:i + 1], bias=nmr[:, i:i + 1])
                else:
                    EV.tensor_scalar(
                        out=ln[:], in0=src_all[:, g0 + i, :],
                        scalar1=mv[:, i, 0:1], scalar2=rstd[:, i:i + 1],
                        op0=ALU.subtract, op1=ALU.mult)
                if ln_w is not None:
                    EV.tensor_tensor(ln[:], ln[:], ln_w[:], op=ALU.mult)
                if ln_b is not None:
                    EV.tensor_tensor(ln[:], ln[:], ln_b[:], op=ALU.add)
                EV.tensor_reduce(
                    out=am[:, i:i + 1], in_=ln[:], axis=AX.X, op=ALU.max,
                    apply_absolute_value=True)
                lns.append(ln)
            amc = tmp.tile([128, G], F32, tag="ln_amc")
            EV.tensor_scalar_max(out=amc[:, :gn], in0=am[:, :gn], scalar1=1e-5)
            qs = tmp.tile([128, G], F32, tag="ln_qs")
            EV.reciprocal(out=qs[:, :gn], in_=amc[:, :gn])
            EV.tensor_scalar_mul(out=qs[:, :gn], in0=qs[:, :gn], scalar1=128.0)
            EV.tensor_scalar_mul(out=rinv[:, g0:g0 + gn], in0=amc[:, :gn],
                                 scalar1=1.0 / 128.0)
            if post_group is not None:
                post_group(rinv, g0, gn)
            qbs = []
            for i in range(gn):
                EG.tensor_scalar(
                    out=lns[i][:], in0=lns[i][:],
                    scalar1=qs[:, i:i + 1], scalar2=MAGIC,
                    op0=ALU.mult, op1=ALU.add)
                qb = tmp.tile([128, DIM], BF16, tag="ln_qb")
                EV.tensor_scalar(
                    out=qb[:], in0=lns[i][:], scalar1=-MAGIC, scalar2=127.0,
                    op0=ALU.add, op1=ALU.min)
                qbs.append(qb)
            return qbs

        # ---------- LN1 + quant (streaming) ----------
        sA_ctx = ExitStack()
        sA = sA_ctx.enter_context(tc.tile_pool(name="sA", bufs=1))

        xkvT_ctx = ExitStack()
        xkvT_pool = xkvT_ctx.enter_context(tc.tile_pool(name="xkvT", bufs=1))
        xkv_qT = xkvT_pool.tile([128, IC, NKV], BF16, tag="xkv_qT")
        dq_kv_qkv = pers.tile([128, CKV], F32, tag="dq_kv_qkv")
        escale = pers.tile([128, CKV], F32, tag="escale")

        with tc.tile_pool(name="xkvP", bufs=1) as xkv_pool:
            xkv_all = xkv_pool.tile([128, CKV, DIM], F32, tag="xkv")
            xkv_r = xkv_d[:].rearrange("(c p) d -> p c d", p=128)
            for j in range(4):
                nc.sync.dma_start(xkv_all[:, j * 4:(j + 1) * 4, :],
                                  xkv_r[:, j * 4:(j + 1) * 4, :])

            def _post_kv(rinv, g0, gn):
                EV.tensor_scalar_mul(out=dq_kv_qkv[:, g0:g0 + gn],
                                     in0=rinv[:, g0:g0 + gn],
                                     scalar1=m_qkv)
                EV.tensor_scalar_mul(out=escale[:, g0:g0 + gn],
                                     in0=rinv[:, g0:g0 + gn],
                                     scalar1=m_qkv * ATT_SCALE)

            r_kv = pers.tile([128, CKV], F32, tag="r_kv")
            with tc.tile_pool(name="lnP1", bufs=2) as ln_pool1:
                for g0 in range(0, CKV, G):
                    kvq = ln_quant_group(xkv_all, g0, G, r_kv, ln_pool1,
                                         ln1_wt, ln1_bt, post_group=_post_kv)
                    for i, qb in enumerate(kvq):
                        c = g0 + i
                        nc.sync.dma_start_transpose(
                            xkv_qT[:, :, c * 128:(c + 1) * 128], qb[:])
        # query tokens are chunks 0..CQ of the (host-reordered) kv sequence
        xq_qT = xkv_qT
        dq_q_qkv = dq_kv_qkv

        # ---------- qkv projections ----------
        v_sb = []
        kT = sA.tile([128, IC, NKV], BF16, tag="kT")
        qT = sA.tile([128, IC, NQ], BF16, tag="qT")
        with tc.tile_pool(name="ps_mm0", bufs=3, space="PSUM") as ps_mm0:
            for c in range(CKV):
                ps = ps_mm0.tile([128, DIM], F32, tag="mm")
                for icx in range(IC):
                    nc.tensor.matmul(
                        ps[:], xkv_qT[:, icx, c * 128:(c + 1) * 128],
                        w_qkv[:, icx, 2 * DIM:3 * DIM],
                        start=(icx == 0), stop=(icx == IC - 1))
                vt = sA.tile([128, HEADS, HD + 1], BF16, tag=f"v{c}")
                ES.activation(
                    out=vt[:, :, 0:HD],
                    in_=ps[:].rearrange("p (h d) -> p h d", h=HEADS),
                    func=AF.Copy, scale=dq_kv_qkv[:, c:c + 1])
                EV.memset(vt[:, :, HD:HD + 1], 1.0)
                v_sb.append(vt)

            # kT integer-valued (token scales folded into the exp scale)
            for mc in range(IC):
                for ns in range(NKV // 512):
                    ps = ps_mm0.tile([128, 512], F32, tag="mm")
                    for icx in range(IC):
                        nc.tensor.matmul(
                            ps[:],
                            w_qkv[:, icx, DIM + mc * 128:DIM + (mc + 1) * 128],
                            xkv_qT[:, icx, ns * 512:(ns + 1) * 512],
                            start=(icx == 0), stop=(icx == IC - 1))
                    EV.tensor_copy(out=kT[:, mc, ns * 512:(ns + 1) * 512], in_=ps[:])

            for c in range(CQ):
                ps = ps_mm0.tile([128, DIM], F32, tag="mm")
                for icx in range(IC):
                    nc.tensor.matmul(
                        ps[:], xq_qT[:, icx, c * 128:(c + 1) * 128],
                        w_qkv[:, icx, 0:DIM],
                        start=(icx == 0), stop=(icx == IC - 1))
                qf = tmp.tile([128, DIM], BF16, tag="q_deq")
                ES.activation(out=qf[:], in_=ps[:], func=AF.Copy,
                              scale=dq_q_qkv[:, c:c + 1])
                nc.sync.dma_start_transpose(
                    qT[:, :, c * 128:(c + 1) * 128], qf[:])

        xkvT_ctx.close()

        # ---------- attention + interleaved MLP ----------
        o_qT = sA.tile([128, IC, NQ], BF16, tag="o_qT")
        r_o_all = pers.tile([128, CQ], F32, tag="r_o")
        dq_o_proj = pers.tile([128, CQ], F32, tag="dq_o_proj")
        x1_all = pers.tile([128, CQ, DIM], F32, tag="x1")
        x2_qT = pers.tile([128, IC, NQ], BF16, tag="x2_qT")
        rinv_x2 = pers.tile([128, CQ], F32, tag="rinv_x2")
        dq_x2_fc1 = pers.tile([128, CQ], F32, tag="dq_x2_fc1")
        dqh = pers.tile([128, CQ], F32, tag="dq_h")

        def _post_x2(rinv, g0, gn):
            EV.tensor_scalar_mul(out=dq_x2_fc1[:, g0:g0 + gn],
                                 in0=rinv[:, g0:g0 + gn],
                                 scalar1=m_fc1)

        with tc.tile_pool(name="attnT", bufs=2) as attn_pool, \
             tc.tile_pool(name="oTs", bufs=1) as oT_pool, \
             tc.tile_pool(name="och", bufs=1) as och_pool, \
             tc.tile_pool(name="lnP3", bufs=1) as ln_pool3, \
             tc.tile_pool(name="hP", bufs=1) as h_pool, \
             tc.tile_pool(name="ps_sc", bufs=2, space="PSUM") as ps_sc, \
             tc.tile_pool(name="ps_oT", bufs=1, space="PSUM") as ps_oT, \
             tc.tile_pool(name="ps_o", bufs=2, space="PSUM") as ps_o:

            def mlp_chunk(c):
                h = h_pool.tile([128, HIDDEN], F32, tag=f"h{c % 3}")
                for nb in range(HIDDEN // 512):
                    psa = ps_o.tile([128, 512], F32, tag="o")
                    for icx in range(IC):
                        nc.tensor.matmul(
                            psa[:],
                            x2_qT[:, icx, c * 128:(c + 1) * 128],
                            w_fc1[:, icx, nb * 512:(nb + 1) * 512],
                            start=(icx == 0), stop=(icx == IC - 1))
                    if fc1_bt is None:
                        ES.activation(out=h[:, nb * 512:(nb + 1) * 512],
                                      in_=psa[:], func=AF.Gelu,
                                      scale=dq_x2_fc1[:, c:c + 1])
                    else:
                        hb = tmp.tile([128, 512], F32, tag="h_bias")
                        EV.tensor_scalar_mul(
                            out=hb[:], in0=psa[:],
                            scalar1=dq_x2_fc1[:, c:c + 1])
                        EV.tensor_tensor(hb[:], hb[:],
                                         fc1_bt[:, nb * 512:(nb + 1) * 512],
                                         op=ALU.add)
                        ES.activation(out=h[:, nb * 512:(nb + 1) * 512],
                                      in_=hb[:], func=AF.Gelu)
                am = tmp.tile([128, 1], F32, tag="h_am")
                EV.tensor_reduce(
                    out=am[:], in_=h[:], axis=AX.X, op=ALU.max,
                    apply_absolute_value=True)
                amc = tmp.tile([128, 1], F32, tag="h_amc")
                EV.tensor_scalar_max(out=amc[:], in0=am[:], scalar1=1e-5)
                qsc = tmp.tile([128, 1], F32, tag=f"h_qsc{c % 3}")
                EV.reciprocal(out=qsc[:], in_=amc[:])
                EV.tensor_scalar_mul(out=qsc[:], in0=qsc[:], scalar1=128.0)
                EV.tensor_scalar(out=dqh[:, c:c + 1], in0=amc[:],
                                 scalar1=1.0 / 128.0, scalar2=m_fc2,
                                 op0=ALU.mult, op1=ALU.mult)
                EG.tensor_scalar(
                    out=h[:], in0=h[:],
                    scalar1=qsc[:], scalar2=MAGIC,
                    op0=ALU.mult, op1=ALU.add)
                hq = h_pool.tile([128, HIDDEN], BF16, tag=f"h_qb{c % 2}")
                EV.tensor_scalar(
                    out=hq[:], in0=h[:], scalar1=-MAGIC, scalar2=127.0,
                    op0=ALU.add, op1=ALU.min)
                hqT = h_pool.tile([128, CH, 128], BF16, tag=f"hqT{c % 3}")
                nc.sync.dma_start_transpose(hqT[:], hq[:])
                ps2 = ps_o.tile([128, DIM], F32, tag="o")
                for icx in range(CH):
                    nc.tensor.matmul(
                        ps2[:], hqT[:, icx, :],
                        w_fc2[:, icx, :],
                        start=(icx == 0), stop=(icx == CH - 1))
                t = tmp.tile([128, DIM], F32, tag="fc2_deq")
                EV.tensor_scalar_mul(out=t[:], in0=ps2[:],
                                     scalar1=dqh[:, c:c + 1])
                if fc2_bt is not None:
                    EV.tensor_tensor(t[:], t[:], fc2_bt[:], op=ALU.add)
                outt = tmp.tile([128, DIM], F32, tag="out_sb")
                EV.tensor_tensor(outt[:], t[:], x1_all[:, c, :], op=ALU.add)
                nc.sync.dma_start(out_d[c * 128:(c + 1) * 128, :], outt[:])

            for qtb in range(NQ // 512):
                oT_tiles = {}
                for hp in range(HEADS // 2):
                    psoT0 = ps_oT.tile([128, 512], F32, tag="oT0")
                    psoT1 = ps_oT.tile([128, 512], F32, tag="oT1")
                    for kc in range(CKV):
                        pssc = ps_sc.tile([128, 1024], F32, tag="sc")
                        nc.tensor.matmul(
                            pssc[:, 0:512],
                            kT[0:64, hp, kc * 128:(kc + 1) * 128],
                            qT[0:64, hp, qtb * 512:(qtb + 1) * 512],
                            start=True, stop=True)
                        nc.tensor.matmul(
                            pssc[:, 512:1024],
                            kT[64:128, hp, kc * 128:(kc + 1) * 128],
                            qT[64:128, hp, qtb * 512:(qtb + 1) * 512],
                            start=True, stop=True)
                        at = attn_pool.tile([128, 1024], BF16, tag="attnT")
                        ES.activation(out=at[:], in_=pssc[:], func=AF.Exp,
                                      scale=escale[:, kc:kc + 1])
                        nc.tensor.matmul(
                            psoT0[0:HD + 1, :], v_sb[kc][:, 2 * hp, :],
                            at[:, 0:512],
                            start=(kc == 0), stop=(kc == CKV - 1))
                        nc.tensor.matmul(
                            psoT1[0:HD + 1, :], v_sb[kc][:, 2 * hp + 1, :],
                            at[:, 512:1024],
                            start=(kc == 0), stop=(kc == CKV - 1))
                    t0 = oT_pool.tile([HD + 1, 512], F32, tag=f"oT{2 * hp}")
                    t1 = oT_pool.tile([HD + 1, 512], F32, tag=f"oT{2 * hp + 1}")
                    EV.tensor_copy(out=t0[:], in_=psoT0[0:HD + 1, :])
                    EV.tensor_copy(out=t1[:], in_=psoT1[0:HD + 1, :])
                    oT_tiles[2 * hp] = t0
                    oT_tiles[2 * hp + 1] = t1

                # stage-split: engine-homogeneous loops so per-chunk
                # cross-engine chains pipeline instead of head-of-line block
                osbs, oreals, qscs, oqs = {}, {}, {}, {}
                for tloc in range(4):
                    pso = ps_o.tile([128, HEADS * (HD + 1)], F32, tag="o")
                    for h in range(HEADS):
                        nc.tensor.transpose(
                            pso[:, h * (HD + 1):(h + 1) * (HD + 1)],
                            oT_tiles[h][:, tloc * 128:(tloc + 1) * 128],
                            eye[0:HD + 1, 0:HD + 1])
                    osb = och_pool.tile([128, HEADS, HD + 1], F32,
                                        tag=f"o_sb{tloc}")
                    EV.tensor_copy(
                        out=osb[:],
                        in_=pso[:].rearrange("p (h d) -> p h d", h=HEADS))
                    osbs[tloc] = osb
                for tloc in range(4):
                    c = qtb * 4 + tloc
                    osb = osbs[tloc]
                    rec = och_pool.tile([128, HEADS], F32, tag="o_rec")
                    EV.reciprocal(out=rec[:], in_=osb[:, :, HD])
                    oreal = och_pool.tile([128, HEADS, HD], F32,
                                          tag=f"o_real{tloc}")
                    EV.tensor_tensor(
                        oreal[:], osb[:, :, 0:HD],
                        rec[:, :, None].to_broadcast((128, HEADS, HD)),
                        op=ALU.mult)
                    am = och_pool.tile([128, 1], F32, tag="o_am")
                    EV.tensor_reduce(
                        out=am[:], in_=oreal[:], axis=AX.XY, op=ALU.max,
                        apply_absolute_value=True)
                    amc = och_pool.tile([128, 1], F32, tag="o_amc")
                    EV.tensor_scalar_max(out=amc[:], in0=am[:], scalar1=1e-5)
                    qsc = och_pool.tile([128, 1], F32, tag=f"o_qsc{tloc}")
                    EV.reciprocal(out=qsc[:], in_=amc[:])
                    EV.tensor_scalar_mul(out=qsc[:], in0=qsc[:], scalar1=128.0)
                    EV.tensor_scalar_mul(
                        out=r_o_all[:, c:c + 1], in0=amc[:], scalar1=1.0 / 128.0)
                    EV.tensor_scalar_mul(
                        out=dq_o_proj[:, c:c + 1], in0=r_o_all[:, c:c + 1],
                        scalar1=m_proj)
                    oreals[tloc] = oreal
                    qscs[tloc] = qsc
                for tloc in range(4):
                    orf = oreals[tloc][:].rearrange("p h d -> p (h d)")
                    EG.tensor_scalar(
                        out=orf, in0=orf,
                        scalar1=qscs[tloc][:], scalar2=MAGIC,
                        op0=ALU.mult, op1=ALU.add)
                    oq = och_pool.tile([128, DIM], BF16, tag=f"o_qb{tloc}")
                    EV.tensor_scalar(
                        out=oq[:], in0=orf, scalar1=-MAGIC, scalar2=127.0,
                        op0=ALU.add, op1=ALU.min)
                    oqs[tloc] = oq
                for tloc in range(4):
                    c = qtb * 4 + tloc
                    nc.sync.dma_start_transpose(
                        o_qT[:, :, c * 128:(c + 1) * 128], oqs[tloc][:])
                for tloc in range(4):
                    c = qtb * 4 + tloc
                    psp = ps_o.tile([128, DIM], F32, tag="o")
                    for icx in range(IC):
                        nc.tensor.matmul(
                            psp[:], o_qT[:, icx, c * 128:(c + 1) * 128],
                            w_proj[:, icx, :],
                            start=(icx == 0), stop=(icx == IC - 1))
                    t2_ = och_pool.tile([128, DIM], F32, tag=f"o_pj{tloc % 2}")
                    EV.tensor_scalar_mul(out=t2_[:], in0=psp[:],
                                         scalar1=dq_o_proj[:, c:c + 1])
                    if proj_bt is not None:
                        EV.tensor_tensor(t2_[:], t2_[:], proj_bt[:], op=ALU.add)
                    xqc = tmp.tile([128, DIM], F32, tag="xq_res")
                    nc.sync.dma_start(xqc[:], xq_d[c * 128:(c + 1) * 128, :])
                    EV.tensor_tensor(x1_all[:, c, :], t2_[:], xqc[:], op=ALU.add)

                # LN2 + quant + MLP for this qtb's 4 chunks (overlaps the
                # next qtb's attention on otherwise-idle engine time)
                x2q = ln_quant_group(x1_all, qtb * 4, 4, rinv_x2, ln_pool3,
                                     ln2_wt, ln2_bt, use_act=False,
                                     post_group=_post_x2)
                for i, qb in enumerate(x2q):
                    c = qtb * 4 + i
                    nc.sync.dma_start_transpose(
                        x2_qT[:, :, c * 128:(c + 1) * 128], qb[:])
                for i in range(4):
                    mlp_chunk(qtb * 4 + i)

        sA_ctx.close()

    nc.compile()
    return nc


_CACHE = {}


def _host_weight_quant(w):
    # ternary (1.58-bit) absmean weight quant; returns (int weights, scale)
    m = float(np.maximum(np.mean(np.abs(w)), 1e-5))
    q = np.clip(np.round(w / m), -1.0, 1.0)
    return q, m


def _meta(inputs):
    meta = {
        "ln1_trivial": bool(np.all(inputs["ln1_w"] == 1) and np.all(inputs["ln1_b"] == 0)),
        "ln2_trivial": bool(np.all(inputs["ln2_w"] == 1) and np.all(inputs["ln2_b"] == 0)),
        "qkv_b_zero": bool(np.all(inputs["qkv_b"] == 0)),
        "proj_b_zero": bool(np.all(inputs["proj_b"] == 0)),
        "fc1_b_zero": bool(np.all(inputs["fc1_b"] == 0)),
        "fc2_b_zero": bool(np.all(inputs["fc2_b"] == 0)),
    }
    for name, key in (("qkv_w", "qkv"), ("proj_w", "proj"),
                      ("fc1_w", "fc1"), ("fc2_w", "fc2")):
        _, m = _host_weight_quant(np.asarray(inputs[name], np.float32))
        meta[f"m_{key}"] = m
    return meta


def build_in_maps(inputs):
    import ml_dtypes

    x = np.ascontiguousarray(inputs["x"], dtype=np.float32)
    meta = _meta(inputs)

    def wq_bf16_T(name):
        q, _ = _host_weight_quant(np.asarray(inputs[name], np.float32))
        return np.ascontiguousarray(q.T.astype(ml_dtypes.bfloat16))

    wqkvT = wq_bf16_T("qkv_w")
    wprojT = wq_bf16_T("proj_w")
    wfc1T = wq_bf16_T("fc1_w")
    wfc2T = wq_bf16_T("fc2_w")
    eye = np.eye(128, dtype=np.float32)

    in_maps = []
    for core in range(8):
        b, half = core // 2, core % 2
        xb = x[b]
        mine = xb[half * 1024:(half + 1) * 1024]
        other = xb[(1 - half) * 1024:(2 - half) * 1024]
        m = {
            "xq": np.ascontiguousarray(mine),
            "xkv": np.ascontiguousarray(np.concatenate([mine, other], axis=0)),
            "wqkvT": wqkvT, "wprojT": wprojT,
            "wfc1T": wfc1T, "wfc2T": wfc2T, "eye": eye,
        }
        if not meta["ln1_trivial"]:
            m["ln1_w"] = np.ascontiguousarray(inputs["ln1_w"], np.float32)
            m["ln1_b"] = np.ascontiguousarray(inputs["ln1_b"], np.float32)
        if not meta["ln2_trivial"]:
            m["ln2_w"] = np.ascontiguousarray(inputs["ln2_w"], np.float32)
            m["ln2_b"] = np.ascontiguousarray(inputs["ln2_b"], np.float32)
        if not meta["proj_b_zero"]:
            m["proj_b"] = np.ascontiguousarray(inputs["proj_b"], np.float32)
        if not meta["fc1_b_zero"]:
            m["fc1_b"] = np.ascontiguousarray(inputs["fc1_b"], np.float32)
        if not meta["fc2_b_zero"]:
            m["fc2_b"] = np.ascontiguousarray(inputs["fc2_b"], np.float32)
        in_maps.append(m)
    return in_maps


def kernel(**inputs):
    x = np.ascontiguousarray(inputs["x"], dtype=np.float32)
    assert x.shape == (4, 2048, 384)
    meta = _meta(inputs)
    key = tuple(sorted(meta.items()))
    if key not in _CACHE:
        _CACHE[key] = build_program(meta)
    nc = _CACHE[key]

    in_maps = build_in_maps(inputs)
    res = run_bass_kernel_spmd(nc, in_maps, core_ids=list(range(8)))
    out = np.empty((4, 2048, 384), dtype=np.float32)
    for core in range(8):
        b, half = core // 2, core % 2
        out[b, half * 1024:(half + 1) * 1024] = res.results[core]["out"]
    return out


if __name__ == "__main__":
    import reference

    inputs = {k: np.asarray(v) for k, v in reference.setup_inputs().items()}
    expected = np.asarray(reference.reference(**inputs))
    actual = kernel(**inputs)
    err = np.linalg.norm(actual - expected) / np.linalg.norm(expected)
    print("Relative error:", err)


# revision 13
# speedup vs baseline: 1.1716x; 1.1716x over previous
"""Trainium2 Bass kernel for nn_Block (BitNet-style quantized transformer block).

Sharding: 8 cores; core c handles batch b=c//2, token half h=c%2 (1024 tokens).
Each core gets the full batch-b tokens (xkv, for K/V) plus its query half (xq),
and replicated host-pre-quantized ternary weights (bf16, exact small ints).
No cross-core communication.

Numerics:
  - weight quant (ternary absmean) is input preprocessing: done host-side,
    exact; per-weight absmean scales are baked into the program as immediates.
  - act_quant produces int8-valued bf16 (exact); activation matmuls are exact
    integer arithmetic accumulated in fp32 PSUM.
  - kT keeps raw int8 K values (token scales folded into the exp scale);
    q is dequantized to real bf16 before scores. Softmax exp runs unshifted
    (scores are O(10), far inside fp32/bf16 range); attn weights and V are
    bf16 (~0.4% rounding, below the model's own int8 quantization noise).
  - round() matches jnp.round exactly (RNE) via the +1.5*2^23 magic trick.

Engine placement (measured: GpSimd tensor_scalar with subtract/min/max ALU ops
hits a ~10-40x slow ucode path and locks the DVE-shared SBUF port, so GpSimd
only runs (mult,add) fast-path quant steps; clamps/casts go to Vector; exp /
gelu / dequant copies to Scalar; LN stats and reduces to Vector).
"""

import sys

sys.path.insert(0, "/opt/trn_rl_repo")

from contextlib import ExitStack

import numpy as np

import concourse.bass as bass
import concourse.bacc as bacc
import concourse.tile as tile
import concourse.mybir as mybir
from concourse.bass_utils import run_bass_kernel_spmd

F32 = mybir.dt.float32
BF16 = mybir.dt.bfloat16
AF = mybir.ActivationFunctionType
ALU = mybir.AluOpType
AX = mybir.AxisListType

DIM = 384
HEADS = 6
HD = 64
HIDDEN = 1536
NKV = 2048
NQ = 1024
CKV = NKV // 128    # 16
CQ = NQ // 128      # 8
IC = DIM // 128     # 3
CH = HIDDEN // 128  # 12
G = 4               # LN/quant group size (chunks)
MAGIC = float(np.float32(1.5 * 2 ** 23))
EPS = 1e-5
ATT_SCALE = HD ** -0.5


def build_program(meta):
    nc = bacc.Bacc("TRN2", target_bir_lowering=False)

    m_qkv = meta["m_qkv"]
    m_proj = meta["m_proj"]
    m_fc1 = meta["m_fc1"]
    m_fc2 = meta["m_fc2"]

    xq_d = nc.dram_tensor("xq", [NQ, DIM], F32, kind="ExternalInput")
    xkv_d = nc.dram_tensor("xkv", [NKV, DIM], F32, kind="ExternalInput")
    wqkvT_d = nc.dram_tensor("wqkvT", [DIM, 3 * DIM], BF16, kind="ExternalInput")
    wprojT_d = nc.dram_tensor("wprojT", [DIM, DIM], BF16, kind="ExternalInput")
    wfc1T_d = nc.dram_tensor("wfc1T", [DIM, HIDDEN], BF16, kind="ExternalInput")
    wfc2T_d = nc.dram_tensor("wfc2T", [HIDDEN, DIM], BF16, kind="ExternalInput")
    eye_d = nc.dram_tensor("eye", [128, 128], F32, kind="ExternalInput")
    out_d = nc.dram_tensor("out", [NQ, DIM], F32, kind="ExternalOutput")

    with tile.TileContext(nc) as tc, ExitStack() as ctx:
        EV, EG, ES = nc.vector, nc.gpsimd, nc.scalar
        pers = ctx.enter_context(tc.tile_pool(name="pers", bufs=1))
        tmp = ctx.enter_context(tc.tile_pool(name="tmp", bufs=3))

        eye = pers.tile([128, 128], F32, tag="eye")
        nc.sync.dma_start(eye[:], eye_d[:])
        eps_t = pers.tile([128, 1], F32, tag="eps")
        EV.memset(eps_t[:], EPS)

        # ---------- pre-quantized ternary weights (bf16, exact ints) -------
        w_qkv = pers.tile([128, IC, 3 * DIM], BF16, tag="w_qkv")
        nc.sync.dma_start(w_qkv[:], wqkvT_d[:].rearrange("(c p) o -> p c o", p=128))
        w_proj = pers.tile([128, IC, DIM], BF16, tag="w_proj")
        nc.sync.dma_start(w_proj[:], wprojT_d[:].rearrange("(c p) o -> p c o", p=128))
        w_fc1 = pers.tile([128, IC, HIDDEN], BF16, tag="w_fc1")
        nc.sync.dma_start(w_fc1[:], wfc1T_d[:].rearrange("(c p) o -> p c o", p=128))
        w_fc2 = pers.tile([128, CH, DIM], BF16, tag="w_fc2")
        nc.sync.dma_start(w_fc2[:], wfc2T_d[:].rearrange("(c p) o -> p c o", p=128))

        # ---------- optional affine/bias tensors (usually compiled out) ----
        def bcast_row(dram_ap, n, name):
            t = pers.tile([128, n], F32, tag=name)
            src = bass.AP(tensor=dram_ap.tensor, offset=dram_ap.offset,
                          ap=[[0, 128]] + list(dram_ap.ap))
            nc.sync.dma_start(t[:], src)
            return t

        ln1_wt = ln1_bt = ln2_wt = ln2_bt = None
        proj_bt = fc1_bt = fc2_bt = None
        if not meta["ln1_trivial"]:
            ln1_wt = bcast_row(nc.dram_tensor("ln1_w", [DIM], F32, kind="ExternalInput")[:], DIM, "ln1w")
            ln1_bt = bcast_row(nc.dram_tensor("ln1_b", [DIM], F32, kind="ExternalInput")[:], DIM, "ln1b")
        if not meta["ln2_trivial"]:
            ln2_wt = bcast_row(nc.dram_tensor("ln2_w", [DIM], F32, kind="ExternalInput")[:], DIM, "ln2w")
            ln2_bt = bcast_row(nc.dram_tensor("ln2_b", [DIM], F32, kind="ExternalInput")[:], DIM, "ln2b")
        assert meta["qkv_b_zero"], "qkv bias not supported in fast path"
        if not meta["proj_b_zero"]:
            proj_bt = bcast_row(nc.dram_tensor("proj_b", [DIM], F32, kind="ExternalInput")[:], DIM, "projb")
        if not meta["fc1_b_zero"]:
            fc1_bt = bcast_row(nc.dram_tensor("fc1_b", [HIDDEN], F32, kind="ExternalInput")[:], HIDDEN, "fc1b")
        if not meta["fc2_b_zero"]:
            fc2_bt = bcast_row(nc.dram_tensor("fc2_b", [DIM], F32, kind="ExternalInput")[:], DIM, "fc2b")

        I32 = mybir.dt.int32
        qmagic = pers.tile([128, 1], I32, tag="qmagic")
        EV.memset(qmagic[:], 0x5F3759DF)

        def rsqrt_dve(out_ap, var_ap, gn):
            # out = 1/sqrt(var + eps) computed entirely on VectorE
            t = tmp.tile([128, G], F32, tag="rs_t")
            EV.tensor_scalar_add(out=t[:, :gn], in0=var_ap, scalar1=EPS)
            y = tmp.tile([128, G], F32, tag="rs_y")
            yi = y[:, :gn].bitcast(I32)
            EV.tensor_scalar(out=yi, in0=t[:, :gn].bitcast(I32),
                             scalar1=1, scalar2=0,
                             op0=ALU.arith_shift_right, op1=ALU.bypass)
            EV.tensor_tensor(yi, qmagic[:, 0:1].to_broadcast((128, gn)), yi,
                             op=ALU.subtract)
            a = tmp.tile([128, G], F32, tag="rs_a")
            for _ in range(3):
                EV.tensor_tensor(a[:, :gn], y[:, :gn], y[:, :gn], op=ALU.mult)
                EV.tensor_tensor(a[:, :gn], a[:, :gn], t[:, :gn], op=ALU.mult)
                EV.tensor_scalar(out=a[:, :gn], in0=a[:, :gn],
                                 scalar1=-0.5, scalar2=1.5,
                                 op0=ALU.mult, op1=ALU.add)
                EV.tensor_tensor(y[:, :gn], y[:, :gn], a[:, :gn], op=ALU.mult)
            EV.tensor_copy(out=out_ap, in_=y[:, :gn])

        # ---------- streaming LN + act_quant (one group of <=G chunks) -----
        def ln_quant_group(src_all, g0, gn, rinv, ln_pool, ln_w, ln_b,
                           use_act=True, post_group=None):
            mv = tmp.tile([128, G, 2], F32, tag="ln_mv")
            for i in range(gn):
                st = tmp.tile([128, 6], F32, tag="ln_bnst")
                EV.bn_stats(out=st[:], in_=src_all[:, g0 + i, :])
                EV.bn_aggr(out=mv[:, i, :], in_=st[:])
            rstd = tmp.tile([128, G], F32, tag="ln_rstd")
            if use_act:
                std = tmp.tile([128, G], F32, tag="ln_std")
                ES.activation(out=std[:, :gn], in_=mv[:, :gn, 1],
                              func=AF.Sqrt, bias=eps_t[:])
                EV.reciprocal(out=rstd[:, :gn], in_=std[:, :gn])
            else:
                rsqrt_dve(rstd[:, :gn], mv[:, :gn, 1], gn)
            nmr = tmp.tile([128, G], F32, tag="ln_nmr")
            EV.tensor_tensor(nmr[:, :gn], mv[:, :gn, 0], rstd[:, :gn],
                             op=ALU.mult)
            EV.tensor_scalar_mul(out=nmr[:, :gn], in0=nmr[:, :gn],
                                 scalar1=-1.0)
            lns = []
            am = tmp.tile([128, G], F32, tag="ln_am")
            for i in range(gn):
                ln = ln_pool.tile([128, DIM], F32, tag=f"ln_{i}")
                if use_act:
                    ES.activation(
                        out=ln[:], in_=src_all[:, g0 + i, :], func=AF.Identity,
                        scale=rstd[:, i:i + 1], bias=nmr[:, i:i + 1])
                else:
                    EV.tensor_scalar(
                        out=ln[:], in0=src_all[:, g0 + i, :],
                        scalar1=mv[:, i, 0:1], scalar2=rstd[:, i:i + 1],
                        op0=ALU.subtract, op1=ALU.mult)
                if ln_w is not None:
                    EV.tensor_tensor(ln[:], ln[:], ln_w[:], op=ALU.mult)
                if ln_b is not None:
                    EV.tensor_tensor(ln[:], ln[:], ln_b[:], op=ALU.add)
                EV.tensor_reduce(
                    out=am[:, i:i + 1], in_=ln[:], axis=AX.X, op=ALU.max,
                    apply_absolute_value=True)
                lns.append(ln)
            amc = tmp.tile([128, G], F32, tag="ln_amc")
            EV.tensor_scalar_max(out=amc[:, :gn], in0=am[:, :gn], scalar1=1e-5)
            qs = tmp.tile([128, G], F32, tag="ln_qs")
            EV.reciprocal(out=qs[:, :gn], in_=amc[:, :gn])
            EV.tensor_scalar_mul(out=qs[:, :gn], in0=qs[:, :gn], scalar1=128.0)
            EV.tensor_scalar_mul(out=rinv[:, g0:g0 + gn], in0=amc[:, :gn],
                                 scalar1=1.0 / 128.0)
            if post_group is not None:
                post_group(rinv, g0, gn)
            qbs = []
            for i in range(gn):
                EG.tensor_scalar(
                    out=lns[i][:], in0=lns[i][:],
                    scalar1=qs[:, i:i + 1], scalar2=MAGIC,
                    op0=ALU.mult, op1=ALU.add)
                qb = tmp.tile([128, DIM], BF16, tag="ln_qb")
                EV.tensor_scalar(
                    out=qb[:], in0=lns[i][:], scalar1=-MAGIC, scalar2=127.0,
                    op0=ALU.add, op1=ALU.min)
                qbs.append(qb)
            return qbs

        # ---------- LN1 + quant (streaming) ----------
        sA_ctx = ExitStack()
        sA = sA_ctx.enter_context(tc.tile_pool(name="sA", bufs=1))

        xkvT_ctx = ExitStack()
        xkvT_pool = xkvT_ctx.enter_context(tc.tile_pool(name="xkvT", bufs=1))
        xkv_qT = xkvT_pool.tile([128, IC, NKV], BF16, tag="xkv_qT")
        dq_kv_qkv = pers.tile([128, CKV], F32, tag="dq_kv_qkv")
        escale = pers.tile([128, CKV], F32, tag="escale")

        with tc.tile_pool(name="xkvP", bufs=1) as xkv_pool:
            xkv_all = xkv_pool.tile([128, CKV, DIM], F32, tag="xkv")
            xkv_r = xkv_d[:].rearrange("(c p) d -> p c d", p=128)
            for j in range(4):
                nc.sync.dma_start(xkv_all[:, j * 4:(j + 1) * 4, :],
                                  xkv_r[:, j * 4:(j + 1) * 4, :])

            def _post_kv(rinv, g0, gn):
                EV.tensor_scalar_mul(out=dq_kv_qkv[:, g0:g0 + gn],
                                     in0=rinv[:, g0:g0 + gn],
                                     scalar1=m_qkv)
                EV.tensor_scalar_mul(out=escale[:, g0:g0 + gn],
                                     in0=rinv[:, g0:g0 + gn],
                                     scalar1=m_qkv * ATT_SCALE)

            r_kv = pers.tile([128, CKV], F32, tag="r_kv")
            with tc.tile_pool(name="lnP1", bufs=2) as ln_pool1:
                for g0 in range(0, CKV, G):
                    kvq = ln_quant_group(xkv_all, g0, G, r_kv, ln_pool1,
                                         ln1_wt, ln1_bt, post_group=_post_kv)
                    for i, qb in enumerate(kvq):
                        c = g0 + i
                        nc.sync.dma_start_transpose(
                            xkv_qT[:, :, c * 128:(c + 1) * 128], qb[:])
        # query tokens are chunks 0..CQ of the (host-reordered) kv sequence
        xq_qT = xkv_qT
        dq_q_qkv = dq_kv_qkv

        # ---------- qkv projections ----------
        v_sb = []
        kT = sA.tile([128, IC, NKV], BF16, tag="kT")
        qT = sA.tile([128, IC, NQ], BF16, tag="qT")
        with tc.tile_pool(name="ps_mm0", bufs=3, space="PSUM") as ps_mm0:
            for c in range(CKV):
                ps = ps_mm0.tile([128, DIM], F32, tag="mm")
                for icx in range(IC):
                    nc.tensor.matmul(
                        ps[:], xkv_qT[:, icx, c * 128:(c + 1) * 128],
                        w_qkv[:, icx, 2 * DIM:3 * DIM],
                        start=(icx == 0), stop=(icx == IC - 1))
                vt = sA.tile([128, HEADS, HD + 1], BF16, tag=f"v{c}")
                EV.tensor_scalar_mul(
                    out=vt[:, :, 0:HD],
                    in0=ps[:].rearrange("p (h d) -> p h d", h=HEADS),
                    scalar1=dq_kv_qkv[:, c:c + 1])
                EV.memset(vt[:, :, HD:HD + 1], 1.0)
                v_sb.append(vt)

            # kT integer-valued (token scales folded into the exp scale)
            for mc in range(IC):
                for ns in range(NKV // 512):
                    ps = ps_mm0.tile([128, 512], F32, tag="mm")
                    for icx in range(IC):
                        nc.tensor.matmul(
                            ps[:],
                            w_qkv[:, icx, DIM + mc * 128:DIM + (mc + 1) * 128],
                            xkv_qT[:, icx, ns * 512:(ns + 1) * 512],
                            start=(icx == 0), stop=(icx == IC - 1))
                    EV.tensor_copy(out=kT[:, mc, ns * 512:(ns + 1) * 512], in_=ps[:])

            for c in range(CQ):
                ps = ps_mm0.tile([128, DIM], F32, tag="mm")
                for icx in range(IC):
                    nc.tensor.matmul(
                        ps[:], xq_qT[:, icx, c * 128:(c + 1) * 128],
                        w_qkv[:, icx, 0:DIM],
                        start=(icx == 0), stop=(icx == IC - 1))
                qf = tmp.tile([128, DIM], BF16, tag="q_deq")
                EV.tensor_scalar_mul(out=qf[:], in0=ps[:],
                                     scalar1=dq_q_qkv[:, c:c + 1])
                nc.sync.dma_start_transpose(
                    qT[:, :, c * 128:(c + 1) * 128], qf[:])

        xkvT_ctx.close()

        # ---------- attention + interleaved MLP ----------
        o_qT = sA.tile([128, IC, NQ], BF16, tag="o_qT")
        r_o_all = pers.tile([128, CQ], F32, tag="r_o")
        dq_o_proj = pers.tile([128, CQ], F32, tag="dq_o_proj")
        x1_all = pers.tile([128, CQ, DIM], F32, tag="x1")
        x2_qT = pers.tile([128, IC, NQ], BF16, tag="x2_qT")
        rinv_x2 = pers.tile([128, CQ], F32, tag="rinv_x2")
        dq_x2_fc1 = pers.tile([128, CQ], F32, tag="dq_x2_fc1")
        dqh = pers.tile([128, CQ], F32, tag="dq_h")
        # raw fc1 accumulator values (ints scaled at gelu time); bf16 keeps
        # them within 0.4% which is below the int8 act-quant noise floor
        h_all = pers.tile([128, CQ, HIDDEN], BF16, tag="h_all")

        def _post_x2(rinv, g0, gn):
            EV.tensor_scalar_mul(out=dq_x2_fc1[:, g0:g0 + gn],
                                 in0=rinv[:, g0:g0 + gn],
                                 scalar1=m_fc1)

        with tc.tile_pool(name="attnT", bufs=2) as attn_pool, \
             tc.tile_pool(name="oTs", bufs=1) as oT_pool, \
             tc.tile_pool(name="och", bufs=1) as och_pool, \
             tc.tile_pool(name="lnP3", bufs=1) as ln_pool3, \
             tc.tile_pool(name="hP", bufs=1) as h_pool, \
             tc.tile_pool(name="ps_sc", bufs=2, space="PSUM") as ps_sc, \
             tc.tile_pool(name="ps_oT", bufs=1, space="PSUM") as ps_oT, \
             tc.tile_pool(name="ps_o", bufs=2, space="PSUM") as ps_o:

            last_exp = [None]

            def mlp_fc1(c):
                # fc1 matmuls + raw-psum evacuation only (no ACT work, so the
                # attention exp table stays loaded); dequant folds into the
                # gelu scale later
                for nb in range(HIDDEN // 512):
                    psa = ps_o.tile([128, 512], F32, tag="o")
                    for icx in range(IC):
                        nc.tensor.matmul(
                            psa[:],
                            x2_qT[:, icx, c * 128:(c + 1) * 128],
                            w_fc1[:, icx, nb * 512:(nb + 1) * 512],
                            start=(icx == 0), stop=(icx == IC - 1))
                    EV.tensor_copy(
                        out=h_all[:, c, nb * 512:(nb + 1) * 512], in_=psa[:])

            def mlp_tail(c):
                import bass_rust

                h = h_pool.tile([128, HIDDEN], F32, tag=f"h{c % 3}")
                if fc1_bt is None:
                    gi = ES.activation(out=h[:], in_=h_all[:, c, :],
                                       func=AF.Gelu,
                                       scale=dq_x2_fc1[:, c:c + 1])
                else:
                    hb = h_pool.tile([128, HIDDEN], F32, tag="h_bias")
                    EV.tensor_scalar_mul(
                        out=hb[:], in0=h_all[:, c, :],
                        scalar1=dq_x2_fc1[:, c:c + 1])
                    EV.tensor_tensor(hb[:], hb[:], fc1_bt[:], op=ALU.add)
                    gi = ES.activation(out=h[:], in_=hb[:], func=AF.Gelu)
                if last_exp[0] is not None:
                    bass_rust.add_dep_helper(
                        gi.ins, last_exp[0].ins, sync=True,
                        reason="keep gelu after all exps (ACT table set)")
                am = tmp.tile([128, 1], F32, tag="h_am")
                EV.tensor_reduce(
                    out=am[:], in_=h[:], axis=AX.X, op=ALU.max,
                    apply_absolute_value=True)
                amc = tmp.tile([128, 1], F32, tag="h_amc")
                EV.tensor_scalar_max(out=amc[:], in0=am[:], scalar1=1e-5)
                qsc = tmp.tile([128, 1], F32, tag=f"h_qsc{c % 3}")
                EV.reciprocal(out=qsc[:], in_=amc[:])
                EV.tensor_scalar_mul(out=qsc[:], in0=qsc[:], scalar1=128.0)
                EV.tensor_scalar(out=dqh[:, c:c + 1], in0=amc[:],
                                 scalar1=1.0 / 128.0, scalar2=m_fc2,
                                 op0=ALU.mult, op1=ALU.mult)
                EG.tensor_scalar(
                    out=h[:], in0=h[:],
                    scalar1=qsc[:], scalar2=MAGIC,
                    op0=ALU.mult, op1=ALU.add)
                hq = h_pool.tile([128, HIDDEN], BF16, tag=f"h_qb{c % 2}")
                EV.tensor_scalar(
                    out=hq[:], in0=h[:], scalar1=-MAGIC, scalar2=127.0,
                    op0=ALU.add, op1=ALU.min)
                hqT = h_pool.tile([128, CH, 128], BF16, tag=f"hqT{c % 3}")
                nc.sync.dma_start_transpose(hqT[:], hq[:])
                ps2 = ps_o.tile([128, DIM], F32, tag="o")
                for icx in range(CH):
                    nc.tensor.matmul(
                        ps2[:], hqT[:, icx, :],
                        w_fc2[:, icx, :],
                        start=(icx == 0), stop=(icx == CH - 1))
                t = tmp.tile([128, DIM], F32, tag="fc2_deq")
                EV.tensor_scalar_mul(out=t[:], in0=ps2[:],
                                     scalar1=dqh[:, c:c + 1])
                if fc2_bt is not None:
                    EV.tensor_tensor(t[:], t[:], fc2_bt[:], op=ALU.add)
                outt = tmp.tile([128, DIM], F32, tag="out_sb")
                EV.tensor_tensor(outt[:], t[:], x1_all[:, c, :], op=ALU.add)
                nc.sync.dma_start(out_d[c * 128:(c + 1) * 128, :], outt[:])

            for qtb in range(NQ // 512):
                oT_tiles = {}
                for hp in range(HEADS // 2):
                    psoT0 = ps_oT.tile([128, 512], F32, tag="oT0")
                    psoT1 = ps_oT.tile([128, 512], F32, tag="oT1")
                    for kc in range(CKV):
                        pssc = ps_sc.tile([128, 1024], F32, tag="sc")
                        nc.tensor.matmul(
                            pssc[:, 0:512],
                            kT[0:64, hp, kc * 128:(kc + 1) * 128],
                            qT[0:64, hp, qtb * 512:(qtb + 1) * 512],
                            start=True, stop=True)
                        nc.tensor.matmul(
                            pssc[:, 512:1024],
                            kT[64:128, hp, kc * 128:(kc + 1) * 128],
                            qT[64:128, hp, qtb * 512:(qtb + 1) * 512],
                            start=True, stop=True)
                        at = attn_pool.tile([128, 1024], BF16, tag="attnT")
                        last_exp[0] = ES.activation(
                            out=at[:], in_=pssc[:], func=AF.Exp,
                            scale=escale[:, kc:kc + 1])
                        nc.tensor.matmul(
                            psoT0[0:HD + 1, :], v_sb[kc][:, 2 * hp, :],
                            at[:, 0:512],
                            start=(kc == 0), stop=(kc == CKV - 1))
                        nc.tensor.matmul(
                            psoT1[0:HD + 1, :], v_sb[kc][:, 2 * hp + 1, :],
                            at[:, 512:1024],
                            start=(kc == 0), stop=(kc == CKV - 1))
                    t0 = oT_pool.tile([HD + 1, 512], F32, tag=f"oT{2 * hp}")
                    t1 = oT_pool.tile([HD + 1, 512], F32, tag=f"oT{2 * hp + 1}")
                    EV.tensor_copy(out=t0[:], in_=psoT0[0:HD + 1, :])
                    EV.tensor_copy(out=t1[:], in_=psoT1[0:HD + 1, :])
                    oT_tiles[2 * hp] = t0
                    oT_tiles[2 * hp + 1] = t1

                # stage-split: engine-homogeneous loops so per-chunk
                # cross-engine chains pipeline instead of head-of-line block
                osbs, oreals, qscs, oqs = {}, {}, {}, {}
                for tloc in range(4):
                    pso = ps_o.tile([128, HEADS * (HD + 1)], F32, tag="o")
                    for h in range(HEADS):
                        nc.tensor.transpose(
                            pso[:, h * (HD + 1):(h + 1) * (HD + 1)],
                            oT_tiles[h][:, tloc * 128:(tloc + 1) * 128],
                            eye[0:HD + 1, 0:HD + 1])
                    osb = och_pool.tile([128, HEADS, HD + 1], F32,
                                        tag=f"o_sb{tloc}")
                    EV.tensor_copy(
                        out=osb[:],
                        in_=pso[:].rearrange("p (h d) -> p h d", h=HEADS))
                    osbs[tloc] = osb
                for tloc in range(4):
                    c = qtb * 4 + tloc
                    osb = osbs[tloc]
                    rec = och_pool.tile([128, HEADS], F32, tag="o_rec")
                    EV.reciprocal(out=rec[:], in_=osb[:, :, HD])
                    oreal = och_pool.tile([128, HEADS, HD], F32,
                                          tag=f"o_real{tloc}")
                    EV.tensor_tensor(
                        oreal[:], osb[:, :, 0:HD],
                        rec[:, :, None].to_broadcast((128, HEADS, HD)),
                        op=ALU.mult)
                    am = och_pool.tile([128, 1], F32, tag="o_am")
                    EV.tensor_reduce(
                        out=am[:], in_=oreal[:], axis=AX.XY, op=ALU.max,
                        apply_absolute_value=True)
                    amc = och_pool.tile([128, 1], F32, tag="o_amc")
                    EV.tensor_scalar_max(out=amc[:], in0=am[:], scalar1=1e-5)
                    qsc = och_pool.tile([128, 1], F32, tag=f"o_qsc{tloc}")
                    EV.reciprocal(out=qsc[:], in_=amc[:])
                    EV.tensor_scalar_mul(out=qsc[:], in0=qsc[:], scalar1=128.0)
                    EV.tensor_scalar_mul(
                        out=r_o_all[:, c:c + 1], in0=amc[:], scalar1=1.0 / 128.0)
                    EV.tensor_scalar_mul(
                        out=dq_o_proj[:, c:c + 1], in0=r_o_all[:, c:c + 1],
                        scalar1=m_proj)
                    oreals[tloc] = oreal
                    qscs[tloc] = qsc
                for tloc in range(4):
                    orf = oreals[tloc][:].rearrange("p h d -> p (h d)")
                    EG.tensor_scalar(
                        out=orf, in0=orf,
                        scalar1=qscs[tloc][:], scalar2=MAGIC,
                        op0=ALU.mult, op1=ALU.add)
                    oq = och_pool.tile([128, DIM], BF16, tag=f"o_qb{tloc}")
                    EV.tensor_scalar(
                        out=oq[:], in0=orf, scalar1=-MAGIC, scalar2=127.0,
                        op0=ALU.add, op1=ALU.min)
                    oqs[tloc] = oq
                for tloc in range(4):
                    c = qtb * 4 + tloc
                    nc.sync.dma_start_transpose(
                        o_qT[:, :, c * 128:(c + 1) * 128], oqs[tloc][:])
                for tloc in range(4):
                    c = qtb * 4 + tloc
                    psp = ps_o.tile([128, DIM], F32, tag="o")
                    for icx in range(IC):
                        nc.tensor.matmul(
                            psp[:], o_qT[:, icx, c * 128:(c + 1) * 128],
                            w_proj[:, icx, :],
                            start=(icx == 0), stop=(icx == IC - 1))
                    t2_ = och_pool.tile([128, DIM], F32, tag=f"o_pj{tloc % 2}")
                    EV.tensor_scalar_mul(out=t2_[:], in0=psp[:],
                                         scalar1=dq_o_proj[:, c:c + 1])
                    if proj_bt is not None:
                        EV.tensor_tensor(t2_[:], t2_[:], proj_bt[:], op=ALU.add)
                    xqc = tmp.tile([128, DIM], F32, tag="xq_res")
                    nc.sync.dma_start(xqc[:], xq_d[c * 128:(c + 1) * 128, :])
                    EV.tensor_tensor(x1_all[:, c, :], t2_[:], xqc[:], op=ALU.add)

                # LN2 + quant + fc1 for this qtb's 4 chunks (overlaps the
                # next qtb's attention on otherwise-idle engine time; nothing
                # here touches an ACT table function)
                x2q = ln_quant_group(x1_all, qtb * 4, 4, rinv_x2, ln_pool3,
                                     ln2_wt, ln2_bt, use_act=False,
                                     post_group=_post_x2)
                for i, qb in enumerate(x2q):
                    c = qtb * 4 + i
                    nc.sync.dma_start_transpose(
                        x2_qT[:, :, c * 128:(c + 1) * 128], qb[:])
                for i in range(4):
                    mlp_fc1(qtb * 4 + i)

            # gelu onward runs after the last exp (single gelu table load)
            for c in range(CQ):
                mlp_tail(c)

        sA_ctx.close()

    nc.compile()
    return nc


_CACHE = {}


def _host_weight_quant(w):
    # ternary (1.58-bit) absmean weight quant; returns (int weights, scale)
    m = float(np.maximum(np.mean(np.abs(w)), 1e-5))
    q = np.clip(np.round(w / m), -1.0, 1.0)
    return q, m


def _meta(inputs):
    meta = {
        "ln1_trivial": bool(np.all(inputs["ln1_w"] == 1) and np.all(inputs["ln1_b"] == 0)),
        "ln2_trivial": bool(np.all(inputs["ln2_w"] == 1) and np.all(inputs["ln2_b"] == 0)),
        "qkv_b_zero": bool(np.all(inputs["qkv_b"] == 0)),
        "proj_b_zero": bool(np.all(inputs["proj_b"] == 0)),
        "fc1_b_zero": bool(np.all(inputs["fc1_b"] == 0)),
        "fc2_b_zero": bool(np.all(inputs["fc2_b"] == 0)),
    }
    for name, key in (("qkv_w", "qkv"), ("proj_w", "proj"),
                      ("fc1_w", "fc1"), ("fc2_w", "fc2")):
        _, m = _host_weight_quant(np.asarray(inputs[name], np.float32))
        meta[f"m_{key}"] = m
    return meta


def build_in_maps(inputs):
    import ml_dtypes

    x = np.ascontiguousarray(inputs["x"], dtype=np.float32)
    meta = _meta(inputs)

    def wq_bf16_T(name):
        q, _ = _host_weight_quant(np.asarray(inputs[name], np.float32))
        return np.ascontiguousarray(q.T.astype(ml_dtypes.bfloat16))

    wqkvT = wq_bf16_T("qkv_w")
    wprojT = wq_bf16_T("proj_w")
    wfc1T = wq_bf16_T("fc1_w")
    wfc2T = wq_bf16_T("fc2_w")
    eye = np.eye(128, dtype=np.float32)

    in_maps = []
    for core in range(8):
        b, half = core // 2, core % 2
        xb = x[b]
        mine = xb[half * 1024:(half + 1) * 1024]
        other = xb[(1 - half) * 1024:(2 - half) * 1024]
        m = {
            "xq": np.ascontiguousarray(mine),
            "xkv": np.ascontiguousarray(np.concatenate([mine, other], axis=0)),
            "wqkvT": wqkvT, "wprojT": wprojT,
            "wfc1T": wfc1T, "wfc2T": wfc2T, "eye": eye,
        }
        if not meta["ln1_trivial"]:
            m["ln1_w"] = np.ascontiguousarray(inputs["ln1_w"], np.float32)
            m["ln1_b"] = np.ascontiguousarray(inputs["ln1_b"], np.float32)
        if not meta["ln2_trivial"]:
            m["ln2_w"] = np.ascontiguousarray(inputs["ln2_w"], np.float32)
            m["ln2_b"] = np.ascontiguousarray(inputs["ln2_b"], np.float32)
        if not meta["proj_b_zero"]:
            m["proj_b"] = np.ascontiguousarray(inputs["proj_b"], np.float32)
        if not meta["fc1_b_zero"]:
            m["fc1_b"] = np.ascontiguousarray(inputs["fc1_b"], np.float32)
        if not meta["fc2_b_zero"]:
            m["fc2_b"] = np.ascontiguousarray(inputs["fc2_b"], np.float32)
        in_maps.append(m)
    return in_maps


def kernel(**inputs):
    x = np.ascontiguousarray(inputs["x"], dtype=np.float32)
    assert x.shape == (4, 2048, 384)
    meta = _meta(inputs)
    key = tuple(sorted(meta.items()))
    if key not in _CACHE:
        _CACHE[key] = build_program(meta)
    nc = _CACHE[key]

    in_maps = build_in_maps(inputs)
    res = run_bass_kernel_spmd(nc, in_maps, core_ids=list(range(8)))
    out = np.empty((4, 2048, 384), dtype=np.float32)
    for core in range(8):
        b, half = core // 2, core % 2
        out[b, half * 1024:(half + 1) * 1024] = res.results[core]["out"]
    return out


if __name__ == "__main__":
    import reference

    inputs = {k: np.asarray(v) for k, v in reference.setup_inputs().items()}
    expected = np.asarray(reference.reference(**inputs))
    actual = kernel(**inputs)
    err = np.linalg.norm(actual - expected) / np.linalg.norm(expected)
    print("Relative error:", err)
